# revision 25
# baseline (speedup 1.0000x reference)
"""Grouped MLP (MoE, 8 experts, SwiGLU) — expert-parallel Bass kernel for 8 TRN2 cores.

Reference computation (per expert e, T=1024 tokens each):
    fc1  = x_e @ w1_e            # [T, 2F]
    gate, val = split(fc1)       # [T, F] each
    act  = silu(gate) * val      # [T, F]
    out  = act @ w2_e            # [T, H]

Sharding: expert-parallel — core e owns expert e entirely. No collectives.

Two-phase per-core layout (v2):
  * Phase 1 (mm1): computes fc1^T per f-block (stationary = w1 block [h,f],
    moving = x^T), swiglu on ACT+DVE, act^T tiles stay resident in SBUF
    (64 x [128,1024] fp16 = 16MB).
  * Phase 2 (mm2): H split into 4 512-col quarters; per quarter, stream w2
    [128,512] tiles and accumulate all 8 token-blocks in 8 PSUM banks over
    all 64 f-blocks, then DMA each [128,512] PSUM tile straight to DRAM.
    w2 read exactly once (32MB), output DMA spread across the phase.
  * w1 DRAM layout gives contiguous 2KB-per-partition DMA lines.
"""

import numpy as np
from contextlib import ExitStack

import concourse.bacc as bacc
import concourse.mybir as mybir
import concourse.tile as tile
from concourse.bass_utils import run_bass_kernel_spmd

E = 8          # experts == cores
T = 1024       # tokens per expert
H = 2048       # hidden
F = 8192       # ffn intermediate (act width)
NHB = H // 128   # 16 h-blocks (contraction blocks for mm1)
NFB = F // 128   # 64 f-blocks (act columns)
NTB = T // 128   # 8 token blocks
HB2 = NHB // 2   # 8 h-blocks per half
NQ = H // 512    # 4 output column quarters

F16 = mybir.dt.float16
F32 = mybir.dt.float32

_CACHE: dict = {}


def build_nc():
    nc = bacc.Bacc(None, target_bir_lowering=False, debug=False, num_devices=E)

    xt_d = nc.declare_dram_parameter("xt", [128, NHB, T], F16, isOutput=False)
    w1_d = nc.declare_dram_parameter("w1t", [2 * NFB, 2, 128, HB2, 128], F16,
                                     isOutput=False)
    w2_d = nc.declare_dram_parameter("w2r", [NFB, 128, H], F16, isOutput=False)
    out_d = nc.declare_dram_parameter("out", [NHB, 128, T], F16, isOutput=True)

    with ExitStack() as ctx:
        tc = ctx.enter_context(tile.TileContext(nc))
        persist = ctx.enter_context(tc.tile_pool(name="persist", bufs=1))
        w1_pool = ctx.enter_context(tc.tile_pool(name="w1", bufs=2))
        silu_pool = ctx.enter_context(tc.tile_pool(name="silu", bufs=2))
        act_pool = ctx.enter_context(tc.tile_pool(name="act", bufs=1))

        prefetched = {}

        def fetch_j(j, qv=None):
            # w1 gate/val blocks, each split into h-halves so the first
            # LDWEIGHTS only waits on a 256KB transfer.  Gate on sync, val
            # on scalar spreads the steady-state w1 stream over both queues.
            if j in prefetched:
                return prefetched.pop(j)
            qv = qv or nc.scalar
            w1g_lo = w1_pool.tile([128, HB2, 128], F16, tag="w1g_lo")
            nc.sync.dma_start(w1g_lo[:], w1_d[j, 0])
            w1g_hi = w1_pool.tile([128, HB2, 128], F16, tag="w1g_hi")
            nc.sync.dma_start(w1g_hi[:], w1_d[j, 1])
            w1v_lo = w1_pool.tile([128, HB2, 128], F16, tag="w1v_lo")
            qv.dma_start(w1v_lo[:], w1_d[NFB + j, 0])
            w1v_hi = w1_pool.tile([128, HB2, 128], F16, tag="w1v_hi")
            qv.dma_start(w1v_hi[:], w1_d[NFB + j, 1])
            return ((w1g_lo, w1g_hi), (w1v_lo, w1v_hi))

        # Only sync+scalar issue DMAs: those queues are hardware-dynamic;
        # the gpsimd queue is software-dynamic (Q7-generated descriptors,
        # ~2-5x slower).  Prime both queues with tiny transfers so their
        # one-time warmup cost is paid before the real loads.
        prime = persist.tile([128, 2, 16], F16, tag="prime")
        nc.sync.dma_start(prime[:, 0, :], xt_d[:, 0, 0:16])
        nc.scalar.dma_start(prime[:, 1, :], xt_d[:, 1, 0:16])

        # Zeroed tile for PE pre-warm matmuls (see phase 1).
        warm = persist.tile([128, 192], F16, tag="warm")
        nc.vector.memset(warm[:], 0.0)

        # xt in four 1MB bundles (8KB contiguous lines) — fewer issues and
        # bigger transfers beat 16 separate tiles while the queues ramp.
        # Sync carries the first gate weights first; scalar starts on xt.
        xtb = [persist.tile([128, 4, T], F16, tag=f"xtb{g}", name=f"xtb{g}")
               for g in range(4)]
        xt = [xtb[h // 4][:, h % 4, :] for h in range(NHB)]
        w1g_lo0 = w1_pool.tile([128, HB2, 128], F16, tag="w1g_lo")
        w1g_hi0 = w1_pool.tile([128, HB2, 128], F16, tag="w1g_hi")
        w1v_lo0 = w1_pool.tile([128, HB2, 128], F16, tag="w1v_lo")
        w1v_hi0 = w1_pool.tile([128, HB2, 128], F16, tag="w1v_hi")

        nc.sync.dma_start(w1g_lo0[:], w1_d[0, 0])
        nc.sync.dma_start(w1g_hi0[:], w1_d[0, 1])
        nc.sync.dma_start(xtb[1][:], xt_d[:, 4:8, :])
        nc.sync.dma_start(xtb[3][:], xt_d[:, 12:16, :])

        nc.scalar.dma_start(xtb[0][:], xt_d[:, 0:4, :])
        nc.scalar.dma_start(xtb[2][:], xt_d[:, 8:12, :])
        nc.scalar.dma_start(w1v_lo0[:], w1_d[NFB, 0])
        nc.scalar.dma_start(w1v_hi0[:], w1_d[NFB, 1])

        prefetched[0] = ((w1g_lo0, w1g_hi0), (w1v_lo0, w1v_hi0))
        prefetched[1] = fetch_j(1)

        act_tiles = []

        # ---- Phase 1: mm1 + swiglu; act^T tiles stay resident ----
        with tc.tile_pool(name="ps1", bufs=2, space="PSUM") as ps1:
            # ~70 tiny matmuls on zeroes keep the PE busy while the first
            # real DMAs land, so the HAM clock-gate reaches 8/8 (2.4GHz)
            # before the first real matmul instead of ~3.4us into it.
            warm_ps = ps1.tile([128, T], F32, tag="gate", name="warm_ps")
            for _ in range(70):
                nc.tensor.matmul(warm_ps[0:64, 0:64], warm[:, 0:64],
                                 warm[:, 64:128], start=True, stop=True)
            for j in range(NFB):
                w1g, w1v = fetch_j(j)
                if j + 1 < NFB:
                    prefetched[j + 1] = fetch_j(j + 1)

                gate_ps = ps1.tile([128, T], F32, tag="gate")
                for h in range(NHB):
                    st, sp = (h == 0), (h == NHB - 1)
                    wt = w1g[h // HB2][:, h % HB2, :]
                    nc.tensor.matmul(gate_ps[:, 0:512], wt, xt[h][:, 0:512],
                                     start=st, stop=sp)
                    nc.tensor.matmul(gate_ps[:, 512:1024], wt,
                                     xt[h][:, 512:1024], start=st, stop=sp)
                val_ps = ps1.tile([128, T], F32, tag="val")
                for h in range(NHB):
                    st, sp = (h == 0), (h == NHB - 1)
                    wt = w1v[h // HB2][:, h % HB2, :]
                    nc.tensor.matmul(val_ps[:, 0:512], wt, xt[h][:, 0:512],
                                     start=st, stop=sp)
                    nc.tensor.matmul(val_ps[:, 512:1024], wt,
                                     xt[h][:, 512:1024], start=st, stop=sp)

                # silu(gate)*val = (sigmoid(gate)*gate)*val; gate-first mul
                # order releases the gate PSUM banks one DVE op earlier
                # (phase 2's first matmuls reuse them).
                sig_sb = silu_pool.tile([128, T], F16, tag="sig")
                nc.scalar.activation(sig_sb[:], gate_ps[:],
                                     mybir.ActivationFunctionType.Sigmoid)
                sg_sb = silu_pool.tile([128, T], F16, tag="sg")
                nc.vector.tensor_mul(sg_sb[:], sig_sb[:], gate_ps[:])
                actt = act_pool.tile([128, T], F16, tag=f"act{j}")
                nc.vector.tensor_mul(actt[:], sg_sb[:], val_ps[:])
                act_tiles.append(actt)

        # ---- Phase 2: mm2, transposed orientation ----
        # Stationary = w2 blocks [128f, 128h]; moving = the resident act^T
        # tiles.  Each stationary feeds 2 N=512 matmuls (token halves), so
        # LDWEIGHTS stays hidden AND w2 is read exactly once (32MB, fits one
        # hardware DMA queue).  Output is out^T per h-block; the host
        # transposes for free.  Chunks of 4 h-blocks: 8 PSUM tiles
        # (4hb x 2 token-halves) accumulate over all 64 f-blocks, then
        # PSUM->SBUF fp16 staging (ACT/DVE half each) and out-DMA on the
        # otherwise-idle scalar queue.
        with tc.tile_pool(name="ps2", bufs=8, space="PSUM") as ps2, \
             tc.tile_pool(name="w2", bufs=4) as w2_pool, \
             tc.tile_pool(name="stage", bufs=4) as stage_pool:
            # Bank permutation: phase-1's last swiglu still reads the
            # odd-buffer gate/val banks (2,3,6,7) when phase 2 starts; touch
            # pool offsets 0,1,4,5 first so the first matmuls don't wait.
            PERM = [0, 1, 4, 5, 2, 3, 6, 7]
            NCH = NHB // 4
            for ch in range(NCH):
                raw = [ps2.tile([128, 512], F32, tag="outp",
                                name=f"outp{ch}_{i}")
                       for i in range(8)]
                outs = [raw[PERM.index(i)] for i in range(8)]
                # In the last chunk, the final 4 f-blocks run h-block-major
                # so each h-block's PSUM tiles close out (and stage+DMA)
                # ~1.7us apart instead of all at the very end.
                last = (ch == NCH - 1)
                jsplit = NFB - 4 if last else NFB
                for j in range(jsplit):
                    w2t = w2_pool.tile([128, 512], F16, tag="w2")
                    nc.sync.dma_start(w2t[:],
                                      w2_d[j, :, ch * 512:(ch + 1) * 512])
                    st, sp = (j == 0), (j == NFB - 1)
                    for hbi in range(4):
                        lhsT = w2t[:, hbi * 128:(hbi + 1) * 128]
                        nc.tensor.matmul(outs[2 * hbi][:], lhsT,
                                         act_tiles[j][:, 0:512],
                                         start=st, stop=sp)
                        nc.tensor.matmul(outs[2 * hbi + 1][:], lhsT,
                                         act_tiles[j][:, 512:1024],
                                         start=st, stop=sp)
                if last:
                    w2last = []
                    for j in range(jsplit, NFB):
                        w2t = w2_pool.tile([128, 512], F16, tag="w2")
                        nc.sync.dma_start(w2t[:],
                                          w2_d[j, :, ch * 512:(ch + 1) * 512])
                        w2last.append(w2t)
                for hbi in range(4):
                    hb = ch * 4 + hbi
                    if last:
                        for jj in range(NFB - jsplit):
                            sp = (jj == NFB - jsplit - 1)
                            lhsT = w2last[jj][:, hbi * 128:(hbi + 1) * 128]
                            nc.tensor.matmul(outs[2 * hbi][:], lhsT,
                                             act_tiles[jsplit + jj][:, 0:512],
                                             start=False, stop=sp)
                            nc.tensor.matmul(outs[2 * hbi + 1][:], lhsT,
                                             act_tiles[jsplit + jj][:, 512:1024],
                                             start=False, stop=sp)
                    stg = stage_pool.tile([128, 1024], F16, tag="stage")
                    nc.scalar.activation(stg[:, 0:512], outs[2 * hbi][:],
                                         mybir.ActivationFunctionType.Copy)
                    nc.vector.tensor_copy(stg[:, 512:1024],
                                          outs[2 * hbi + 1][:])
                    q_out = nc.sync if last else nc.scalar
                    q_out.dma_start(out_d[hb], stg[:])

    nc.compile()
    return nc


def _get_nc():
    if "nc" not in _CACHE:
        _CACHE["nc"] = build_nc()
    return _CACHE["nc"]


def prep_inputs(permuted_hidden_states, w1, w2):
    """Host-side reshape/cast into the per-core DMA-friendly layouts."""
    x = np.asarray(permuted_hidden_states, dtype=np.float32)
    w1 = np.asarray(w1, dtype=np.float32)
    w2 = np.asarray(w2, dtype=np.float32)

    # xt[e][p, hb, t] = x[e*T + t, hb*128 + p]
    xt = np.ascontiguousarray(
        x.reshape(E, T, NHB, 128).transpose(0, 3, 2, 1).astype(np.float16))
    # w1t[e][jg, half, p, hb2, fi] = w1[e, (half*HB2+hb2)*128 + p, jg*128 + fi]
    w1t = np.ascontiguousarray(
        w1.reshape(E, 2, HB2, 128, 2 * NFB, 128)
          .transpose(0, 4, 1, 3, 2, 5).astype(np.float16))
    # w2r[e][j, p, :] = w2[e, j*128 + p, :]
    w2r = np.ascontiguousarray(w2.reshape(E, NFB, 128, H).astype(np.float16))
    return xt, w1t, w2r


def run_cores(inputs, trace=False, **spmd_kwargs):
    xt, w1t, w2r = prep_inputs(
        inputs["permuted_hidden_states"], inputs["w1"], inputs["w2"])
    nc = _get_nc()
    in_maps = [{"xt": xt[e], "w1t": w1t[e], "w2r": w2r[e]} for e in range(E)]
    res = run_bass_kernel_spmd(nc, in_maps, list(range(E)), trace=trace, **spmd_kwargs)
    outs = [
        res.results[e]["out"].reshape(NHB, 128, T).transpose(2, 0, 1).reshape(T, H)
        for e in range(E)
    ]
    full = np.concatenate(outs, axis=0).astype(np.float32)
    return full, res


def kernel(permuted_hidden_states, tokens_per_expert, w1, w2):
    full, _ = run_cores({
        "permuted_hidden_states": permuted_hidden_states,
        "w1": w1,
        "w2": w2,
    })
    return full


# revision 26
# speedup vs baseline: 1.0019x; 1.0019x over previous
"""Grouped MLP (MoE, 8 experts, SwiGLU) — expert-parallel Bass kernel for 8 TRN2 cores.

Reference computation (per expert e, T=1024 tokens each):
    fc1  = x_e @ w1_e            # [T, 2F]
    gate, val = split(fc1)       # [T, F] each
    act  = silu(gate) * val      # [T, F]
    out  = act @ w2_e            # [T, H]

Sharding: expert-parallel — core e owns expert e entirely. No collectives.

Two-phase per-core layout (v2):
  * Phase 1 (mm1): computes fc1^T per f-block (stationary = w1 block [h,f],
    moving = x^T), swiglu on ACT+DVE, act^T tiles stay resident in SBUF
    (64 x [128,1024] fp16 = 16MB).
  * Phase 2 (mm2): H split into 4 512-col quarters; per quarter, stream w2
    [128,512] tiles and accumulate all 8 token-blocks in 8 PSUM banks over
    all 64 f-blocks, then DMA each [128,512] PSUM tile straight to DRAM.
    w2 read exactly once (32MB), output DMA spread across the phase.
  * w1 DRAM layout gives contiguous 2KB-per-partition DMA lines.
"""

import numpy as np
from contextlib import ExitStack

import concourse.bacc as bacc
import concourse.mybir as mybir
import concourse.tile as tile
from concourse.bass_utils import run_bass_kernel_spmd

E = 8          # experts == cores
T = 1024       # tokens per expert
H = 2048       # hidden
F = 8192       # ffn intermediate (act width)
NHB = H // 128   # 16 h-blocks (contraction blocks for mm1)
NFB = F // 128   # 64 f-blocks (act columns)
NTB = T // 128   # 8 token blocks
HB2 = NHB // 2   # 8 h-blocks per half
NQ = H // 512    # 4 output column quarters

F16 = mybir.dt.float16
F32 = mybir.dt.float32

_CACHE: dict = {}


def build_nc():
    nc = bacc.Bacc(None, target_bir_lowering=False, debug=False, num_devices=E)

    xt_d = nc.declare_dram_parameter("xt", [128, NHB, T], F16, isOutput=False)
    w1_d = nc.declare_dram_parameter("w1t", [2 * NFB, 2, 128, HB2, 128], F16,
                                     isOutput=False)
    w2_d = nc.declare_dram_parameter("w2r", [NFB, 128, H], F16, isOutput=False)
    out_d = nc.declare_dram_parameter("out", [NHB, 128, T], F16, isOutput=True)

    with ExitStack() as ctx:
        tc = ctx.enter_context(tile.TileContext(nc))
        persist = ctx.enter_context(tc.tile_pool(name="persist", bufs=1))
        w1_pool = ctx.enter_context(tc.tile_pool(name="w1", bufs=2))
        silu_pool = ctx.enter_context(tc.tile_pool(name="silu", bufs=2))
        act_pool = ctx.enter_context(tc.tile_pool(name="act", bufs=1))

        prefetched = {}

        def fetch_j(j, qv=None):
            # w1 gate/val blocks, each split into h-halves so the first
            # LDWEIGHTS only waits on a 256KB transfer.  Gate on sync, val
            # on scalar spreads the steady-state w1 stream over both queues.
            if j in prefetched:
                return prefetched.pop(j)
            qv = qv or nc.scalar
            w1g_lo = w1_pool.tile([128, HB2, 128], F16, tag="w1g_lo")
            nc.sync.dma_start(w1g_lo[:], w1_d[j, 0])
            w1g_hi = w1_pool.tile([128, HB2, 128], F16, tag="w1g_hi")
            nc.sync.dma_start(w1g_hi[:], w1_d[j, 1])
            w1v_lo = w1_pool.tile([128, HB2, 128], F16, tag="w1v_lo")
            qv.dma_start(w1v_lo[:], w1_d[NFB + j, 0])
            w1v_hi = w1_pool.tile([128, HB2, 128], F16, tag="w1v_hi")
            qv.dma_start(w1v_hi[:], w1_d[NFB + j, 1])
            return ((w1g_lo, w1g_hi), (w1v_lo, w1v_hi))

        # Only sync+scalar issue DMAs: those queues are hardware-dynamic;
        # the gpsimd queue is software-dynamic (Q7-generated descriptors,
        # ~2-5x slower).  Prime both queues with tiny transfers so their
        # one-time warmup cost is paid before the real loads.
        prime = persist.tile([128, 2, 16], F16, tag="prime")
        nc.sync.dma_start(prime[:, 0, :], xt_d[:, 0, 0:16])
        nc.scalar.dma_start(prime[:, 1, :], xt_d[:, 1, 0:16])

        # Zeroed tile for PE pre-warm matmuls (see phase 1).
        warm = persist.tile([128, 192], F16, tag="warm")
        nc.vector.memset(warm[:], 0.0)

        # First gate weights ahead of everything on sync; xt split across
        # both queues (evens on scalar so xt0 doesn't queue behind w1g).
        w1g_lo0 = w1_pool.tile([128, HB2, 128], F16, tag="w1g_lo")
        nc.sync.dma_start(w1g_lo0[:], w1_d[0, 0])
        w1g_hi0 = w1_pool.tile([128, HB2, 128], F16, tag="w1g_hi")
        nc.sync.dma_start(w1g_hi0[:], w1_d[0, 1])

        xt = []
        for h in range(NHB):
            xh = persist.tile([128, T], F16, tag=f"xt{h}")
            qx = nc.scalar if h % 2 == 0 else nc.sync
            qx.dma_start(xh[:], xt_d[:, h, :])
            xt.append(xh)

        w1v_lo0 = w1_pool.tile([128, HB2, 128], F16, tag="w1v_lo")
        nc.scalar.dma_start(w1v_lo0[:], w1_d[NFB, 0])
        w1v_hi0 = w1_pool.tile([128, HB2, 128], F16, tag="w1v_hi")
        nc.scalar.dma_start(w1v_hi0[:], w1_d[NFB, 1])
        prefetched[0] = ((w1g_lo0, w1g_hi0), (w1v_lo0, w1v_hi0))
        prefetched[1] = fetch_j(1)

        act_tiles = []

        # ---- Phase 1: mm1 + swiglu; act^T tiles stay resident ----
        with tc.tile_pool(name="ps1", bufs=2, space="PSUM") as ps1:
            # ~70 tiny matmuls on zeroes keep the PE busy while the first
            # real DMAs land, so the HAM clock-gate reaches 8/8 (2.4GHz)
            # before the first real matmul instead of ~3.4us into it.
            warm_ps = ps1.tile([128, T], F32, tag="gate", name="warm_ps")
            for _ in range(70):
                nc.tensor.matmul(warm_ps[0:64, 0:64], warm[:, 0:64],
                                 warm[:, 64:128], start=True, stop=True)
            for j in range(NFB):
                w1g, w1v = fetch_j(j)
                if j + 1 < NFB:
                    prefetched[j + 1] = fetch_j(j + 1)

                gate_ps = ps1.tile([128, T], F32, tag="gate")
                for h in range(NHB):
                    st, sp = (h == 0), (h == NHB - 1)
                    wt = w1g[h // HB2][:, h % HB2, :]
                    nc.tensor.matmul(gate_ps[:, 0:512], wt, xt[h][:, 0:512],
                                     start=st, stop=sp)
                    nc.tensor.matmul(gate_ps[:, 512:1024], wt,
                                     xt[h][:, 512:1024], start=st, stop=sp)
                val_ps = ps1.tile([128, T], F32, tag="val")
                for h in range(NHB):
                    st, sp = (h == 0), (h == NHB - 1)
                    wt = w1v[h // HB2][:, h % HB2, :]
                    nc.tensor.matmul(val_ps[:, 0:512], wt, xt[h][:, 0:512],
                                     start=st, stop=sp)
                    nc.tensor.matmul(val_ps[:, 512:1024], wt,
                                     xt[h][:, 512:1024], start=st, stop=sp)

                # silu(gate)*val = (sigmoid(gate)*gate)*val; gate-first mul
                # order releases the gate PSUM banks one DVE op earlier
                # (phase 2's first matmuls reuse them).
                sig_sb = silu_pool.tile([128, T], F16, tag="sig")
                nc.scalar.activation(sig_sb[:], gate_ps[:],
                                     mybir.ActivationFunctionType.Sigmoid)
                sg_sb = silu_pool.tile([128, T], F16, tag="sg")
                nc.vector.tensor_mul(sg_sb[:], sig_sb[:], gate_ps[:])
                actt = act_pool.tile([128, T], F16, tag=f"act{j}")
                nc.vector.tensor_mul(actt[:], sg_sb[:], val_ps[:])
                act_tiles.append(actt)

        # ---- Phase 2: mm2, transposed orientation ----
        # Stationary = w2 blocks [128f, 128h]; moving = the resident act^T
        # tiles.  Each stationary feeds 2 N=512 matmuls (token halves), so
        # LDWEIGHTS stays hidden AND w2 is read exactly once (32MB, fits one
        # hardware DMA queue).  Output is out^T per h-block; the host
        # transposes for free.  Chunks of 4 h-blocks: 8 PSUM tiles
        # (4hb x 2 token-halves) accumulate over all 64 f-blocks, then
        # PSUM->SBUF fp16 staging (ACT/DVE half each) and out-DMA on the
        # otherwise-idle scalar queue.
        with tc.tile_pool(name="ps2", bufs=8, space="PSUM") as ps2, \
             tc.tile_pool(name="w2", bufs=4) as w2_pool, \
             tc.tile_pool(name="stage", bufs=4) as stage_pool:
            # Bank permutation: phase-1's last swiglu still reads the
            # odd-buffer gate/val banks (2,3,6,7) when phase 2 starts; touch
            # pool offsets 0,1,4,5 first so the first matmuls don't wait.
            PERM = [0, 1, 4, 5, 2, 3, 6, 7]
            NCH = NHB // 4
            for ch in range(NCH):
                raw = [ps2.tile([128, 512], F32, tag="outp",
                                name=f"outp{ch}_{i}")
                       for i in range(8)]
                outs = [raw[PERM.index(i)] for i in range(8)]
                # In the last chunk, the final 4 f-blocks run h-block-major
                # so each h-block's PSUM tiles close out (and stage+DMA)
                # ~1.7us apart instead of all at the very end.
                last = (ch == NCH - 1)
                jsplit = NFB - 4 if last else NFB
                for j in range(jsplit):
                    w2t = w2_pool.tile([128, 512], F16, tag="w2")
                    nc.sync.dma_start(w2t[:],
                                      w2_d[j, :, ch * 512:(ch + 1) * 512])
                    st, sp = (j == 0), (j == NFB - 1)
                    for hbi in range(4):
                        lhsT = w2t[:, hbi * 128:(hbi + 1) * 128]
                        nc.tensor.matmul(outs[2 * hbi][:], lhsT,
                                         act_tiles[j][:, 0:512],
                                         start=st, stop=sp)
                        nc.tensor.matmul(outs[2 * hbi + 1][:], lhsT,
                                         act_tiles[j][:, 512:1024],
                                         start=st, stop=sp)
                if last:
                    w2last = []
                    for j in range(jsplit, NFB):
                        w2t = w2_pool.tile([128, 512], F16, tag="w2")
                        nc.sync.dma_start(w2t[:],
                                          w2_d[j, :, ch * 512:(ch + 1) * 512])
                        w2last.append(w2t)
                for hbi in range(4):
                    hb = ch * 4 + hbi
                    if last:
                        for jj in range(NFB - jsplit):
                            sp = (jj == NFB - jsplit - 1)
                            lhsT = w2last[jj][:, hbi * 128:(hbi + 1) * 128]
                            nc.tensor.matmul(outs[2 * hbi][:], lhsT,
                                             act_tiles[jsplit + jj][:, 0:512],
                                             start=False, stop=sp)
                            nc.tensor.matmul(outs[2 * hbi + 1][:], lhsT,
                                             act_tiles[jsplit + jj][:, 512:1024],
                                             start=False, stop=sp)
                    stg = stage_pool.tile([128, 1024], F16, tag="stage")
                    nc.scalar.activation(stg[:, 0:512], outs[2 * hbi][:],
                                         mybir.ActivationFunctionType.Copy)
                    nc.vector.tensor_copy(stg[:, 512:1024],
                                          outs[2 * hbi + 1][:])
                    q_out = nc.sync if last else nc.scalar
                    q_out.dma_start(out_d[hb], stg[:])

    nc.compile()
    return nc


def _get_nc():
    if "nc" not in _CACHE:
        _CACHE["nc"] = build_nc()
    return _CACHE["nc"]


def prep_inputs(permuted_hidden_states, w1, w2):
    """Host-side reshape/cast into the per-core DMA-friendly layouts."""
    x = np.asarray(permuted_hidden_states, dtype=np.float32)
    w1 = np.asarray(w1, dtype=np.float32)
    w2 = np.asarray(w2, dtype=np.float32)

    # xt[e][p, hb, t] = x[e*T + t, hb*128 + p]
    xt = np.ascontiguousarray(
        x.reshape(E, T, NHB, 128).transpose(0, 3, 2, 1).astype(np.float16))
    # w1t[e][jg, half, p, hb2, fi] = w1[e, (half*HB2+hb2)*128 + p, jg*128 + fi]
    w1t = np.ascontiguousarray(
        w1.reshape(E, 2, HB2, 128, 2 * NFB, 128)
          .transpose(0, 4, 1, 3, 2, 5).astype(np.float16))
    # w2r[e][j, p, :] = w2[e, j*128 + p, :]
    w2r = np.ascontiguousarray(w2.reshape(E, NFB, 128, H).astype(np.float16))
    return xt, w1t, w2r


def run_cores(inputs, trace=False, **spmd_kwargs):
    xt, w1t, w2r = prep_inputs(
        inputs["permuted_hidden_states"], inputs["w1"], inputs["w2"])
    nc = _get_nc()
    in_maps = [{"xt": xt[e], "w1t": w1t[e], "w2r": w2r[e]} for e in range(E)]
    res = run_bass_kernel_spmd(nc, in_maps, list(range(E)), trace=trace, **spmd_kwargs)
    outs = [
        res.results[e]["out"].reshape(NHB, 128, T).transpose(2, 0, 1).reshape(T, H)
        for e in range(E)
    ]
    full = np.concatenate(outs, axis=0).astype(np.float32)
    return full, res


def kernel(permuted_hidden_states, tokens_per_expert, w1, w2):
    full, _ = run_cores({
        "permuted_hidden_states": permuted_hidden_states,
        "w1": w1,
        "w2": w2,
    })
    return full
